# revision 1
# baseline (speedup 1.0000x reference)
"""AttentionQualifierAggregation kernel for 8 trn2 NeuronCores.

Strategy (v1): shard the 500k qualifier rows across the 8 cores; each core
projects its x_q shard, computes per-head attention logits against gathered
per-edge dot-products (beta trick: the edge side of the logit is a per-edge
scalar per head, so the 256-wide x_edge gather collapses to 4 floats), and
scatter-adds softmax-weighted projections into per-edge partials which are
summed across cores.

The numerically-stable segment max is algebraically unnecessary here (logits
are O(1): exp never overflows), so the softmax is computed as
sum(exp(a)*v)/sum(exp(a)) per segment, with the reference's +1e-16 epsilon
on the denominator reproduced exactly (empty segments -> 0).
"""

import numpy as np

NUM_Q = 500000
NUM_E = 250000
DIM = 256
HEADS = 4
DHEAD = DIM // HEADS
NEG_SLOPE = 0.01
EPS = 1e-16
N_CORES = 8


def _device_impl(x_q, x_edge, w_q, weight, edge_ids):
    import jax
    import jax.numpy as jnp
    from jax.sharding import Mesh, PartitionSpec as P
    from jax.experimental.shard_map import shard_map

    devs = jax.devices()[:N_CORES]
    mesh = Mesh(np.asarray(devs), ("q",))

    w_qj = jnp.asarray(w_q)
    weightj = jnp.asarray(weight)
    # beta[e, h] = x_edge[e].reshape(H, DHEAD)[h] . weight[h, :DHEAD]
    # computed sharded over edges, allgathered (tiny: NUM_E x 4).
    def beta_body(xe):
        return jnp.einsum("ehd,hd->eh", xe.reshape(-1, HEADS, DHEAD),
                          weightj[:, :DHEAD])

    beta = shard_map(
        lambda xe: jax.lax.all_gather(beta_body(xe), "q", axis=0, tiled=True),
        mesh=mesh, in_specs=P("q"), out_specs=P(),
    )(jnp.asarray(x_edge))

    wg = weightj[:, DHEAD:]  # (H, DHEAD)

    def body(xq_shard, eid_shard, beta_full):
        proj = xq_shard @ w_qj                                  # (nq, DIM)
        gamma = jnp.einsum("qhd,hd->qh", proj.reshape(-1, HEADS, DHEAD), wg)
        alpha = beta_full[eid_shard] + gamma                    # (nq, H)
        alpha = jnp.where(alpha >= 0, alpha, NEG_SLOPE * alpha)
        ex = jnp.exp(alpha)                                     # (nq, H)
        num = jax.ops.segment_sum(
            (ex[:, :, None] * proj.reshape(-1, HEADS, DHEAD)).reshape(-1, DIM),
            eid_shard, num_segments=NUM_E)
        den = jax.ops.segment_sum(ex, eid_shard, num_segments=NUM_E)
        num = jax.lax.psum(num, "q")
        den = jax.lax.psum(den, "q")
        out = num.reshape(NUM_E, HEADS, DHEAD) / (den[:, :, None] + EPS)
        return out.reshape(NUM_E, DIM)

    fn = shard_map(body, mesh=mesh,
                   in_specs=(P("q"), P("q"), P()),
                   out_specs=P())
    out = fn(jnp.asarray(x_q), jnp.asarray(edge_ids.astype(np.int32)),
             beta)
    return np.asarray(jax.device_get(out)).astype(np.float32)


def _host_impl(x_q, x_edge, w_q, weight, edge_ids):
    x_q = np.asarray(x_q, np.float32)
    x_edge = np.asarray(x_edge, np.float32)
    w_q = np.asarray(w_q, np.float32)
    weight = np.asarray(weight, np.float32)
    eid = np.asarray(edge_ids).astype(np.int64)

    proj = x_q @ w_q
    gamma = np.einsum("qhd,hd->qh",
                      proj.reshape(-1, HEADS, DHEAD), weight[:, DHEAD:])
    beta = np.einsum("ehd,hd->eh",
                     x_edge.reshape(-1, HEADS, DHEAD), weight[:, :DHEAD])
    alpha = beta[eid] + gamma
    alpha = np.where(alpha >= 0, alpha, NEG_SLOPE * alpha).astype(np.float32)
    ex = np.exp(alpha)
    num = np.zeros((NUM_E, DIM), np.float32)
    wq_rows = (ex[:, :, None] * proj.reshape(-1, HEADS, DHEAD)).reshape(-1, DIM)
    np.add.at(num, eid, wq_rows)
    den = np.zeros((NUM_E, HEADS), np.float32)
    np.add.at(den, eid, ex)
    out = num.reshape(NUM_E, HEADS, DHEAD) / (den[:, :, None] + EPS)
    return out.reshape(NUM_E, DIM).astype(np.float32)


def kernel(x_q, x_edge, w_q, weight, edge_ids):
    try:
        return _device_impl(x_q, x_edge, w_q, weight, edge_ids)
    except Exception as e:  # pragma: no cover - device fallback
        import sys
        print(f"kernel: device path failed ({type(e).__name__}: {e}); "
              f"falling back to host", file=sys.stderr)
        return _host_impl(x_q, x_edge, w_q, weight, edge_ids)

